# revision 1
# baseline (speedup 1.0000x reference)
"""ADM attention block (B=4, C=512, H=W=64) on 8 TRN2 NeuronCores.

Sharding: core = (b, half) = (core//2, core%2). Data-parallel over batch (4)
x query-halves (2). Zero collectives: each core computes the full QKV for its
batch sample (k, v needed in full anyway), then attention + output projection
for its half of the queries. The query half is selected purely on the host by
permuting the N axis of x so "my" queries are always columns 0:2048 (SPMD
cores run an identical graph; only inputs differ).

Per-core device algorithm:
  phase 0: weight-norm w = g * v / ||v_row|| folded as a column scale of v^T
           (ones-matmul column sumsq -> s = g*rsqrt -> DMA roundtrip
           broadcast across partitions -> scale, writing bf16 weights).
  phase 1: qkvT[n,3C] = x^T w_qkv^T, 128 n-rows at a time (PSUM [128,3x512]);
           RMS over 3C is a free-axis ACT square+accum; 1/(sqrt(mean)+1e-4);
           v-section is normalized straight into persistent v_sb[j,c'] (the
           attention rhs needs exactly this layout -- no transpose); q,k
           sections are normalized to bf16 then PE-transposed to [c,n].
  phase 2: per 256-query i-slice: scoresT[j,i] = k^T q per j-chunk;
           ex = exp(scores * C^-0.5) on ACT straight from PSUM; h[i,c'] and
           den[i] accumulate over j in PSUM (den via ones-column matmuls);
           epilogue: h *= 2^-0.5/den, PE-transpose to [c',i], project through
           w_out^T, add the (host-prescaled) residual, DMA out.

All matmuls/transposes run in bf16 (1 cycle/row, FWL weight loads); PSUM
accumulation is fp32; RMS/softmax denominators stay fp32.
"""

import os
from contextlib import ExitStack

import numpy as np
import ml_dtypes

import concourse.bass as bass
import concourse.mybir as mybir
import concourse.tile as tile
from concourse.bass_utils import run_bass_kernel_spmd

B, C, N = 4, 512, 4096
NH = N // 2
P = 128
KC = C // P            # 4 c-chunks
NCH = N // P           # 32 n-chunks
QCH = NH // P          # 16 query chunks per core
O3 = 3 * C             # 1536
F32 = mybir.dt.float32
F32R = mybir.dt.float32r
BF16 = mybir.dt.bfloat16
ISL = 512              # query i-slice
NISL = NH // ISL       # 4 i-slices

LAST_RESULT = None

_TPB_ENGINES = (
    mybir.EngineType.PE,
    mybir.EngineType.Activation,
    mybir.EngineType.DVE,
    mybir.EngineType.Pool,
    mybir.EngineType.SP,
)


def _split_waits(nc):
    """walrus on this image rejects >1 sem-wait on a TPB instruction (f32r
    matmul LDW lowering; tail Drain etc). Hoist excess waits onto engine-local
    NoOps, each carrying one wait -- semantically identical, waits run in
    queue order before the instruction."""
    ctr = 0
    for fn in nc.m.functions:
        for blk in fn.blocks:
            new_insts = []
            for inst in blk.instructions:
                si = getattr(inst, "sync_info", None)
                eng = getattr(inst, "engine", None)
                if (
                    si is not None
                    and si.on_wait
                    and len(si.on_wait) > 1
                    and eng in _TPB_ENGINES
                ):
                    for sw in si.on_wait[:-1]:
                        ctr += 1
                        nop = mybir.InstNoOp(
                            name=f"wsplit-{ctr}", engine=eng, ins=[], outs=[],
                            sync_info=mybir.SyncInfo(on_wait=[sw], on_update=[]),
                        )
                        new_insts.append(nop)
                    inst.sync_info = mybir.SyncInfo(
                        on_wait=[si.on_wait[-1]], on_update=si.on_update,
                    )
                new_insts.append(inst)
            blk.instructions[:] = new_insts


def build_graph():
    nc = bass.Bass()

    x_bf_d = nc.declare_dram_parameter("x_bf", [C, N], BF16, isOutput=False)
    xt_nc = nc.declare_dram_parameter("xt_nc", [NH, C], F32, isOutput=False)
    wqkvT_d = nc.declare_dram_parameter("wqkvT", [C, O3], BF16, isOutput=False)
    g_qkv_d = nc.declare_dram_parameter("g_qkv", [1, O3], F32, isOutput=False)
    woutT_d = nc.declare_dram_parameter("woutT", [C, C], BF16, isOutput=False)
    g_out_d = nc.declare_dram_parameter("g_out", [1, C], F32, isOutput=False)
    ident_d = nc.declare_dram_parameter("ident", [P, P], BF16, isOutput=False)
    identf_d = nc.declare_dram_parameter("identf", [P, P], F32, isOutput=False)
    ones_d = nc.declare_dram_parameter("ones_col", [P, 1], F32, isOutput=False)
    out_d = nc.declare_dram_parameter("out", [NH, C], F32, isOutput=True)

    with tile.TileContext(nc) as tc, ExitStack() as ctx:
        singles = ctx.enter_context(tc.tile_pool(name="singles", bufs=1))

        wq_bf = singles.tile([P, KC, O3], BF16)
        nc.sync.dma_start(out=wq_bf, in_=wqkvT_d[:, :].rearrange("(k p) o -> p k o", p=P))
        wo_bf = singles.tile([P, KC, C], BF16)
        nc.sync.dma_start(out=wo_bf, in_=woutT_d[:, :].rearrange("(k p) o -> p k o", p=P))
        ident = singles.tile([P, P], BF16)
        nc.sync.dma_start(out=ident, in_=ident_d[:, :])
        identf = singles.tile([P, P], F32)
        nc.sync.dma_start(out=identf, in_=identf_d[:, :])
        ones_mat = singles.tile([P, P], BF16)
        nc.vector.memset(ones_mat, 1.0)
        ones_bf = singles.tile([P, 1], BF16)
        nc.vector.memset(ones_bf, 1.0)
        ones_f = singles.tile([P, 1], F32R)
        nc.sync.dma_start(out=ones_f, in_=ones_d[:, :].bitcast(F32R))
        g_sb = singles.tile([1, O3], F32)
        nc.sync.dma_start(out=g_sb, in_=g_qkv_d[:, :])
        go_sb = singles.tile([1, C], F32)
        nc.sync.dma_start(out=go_sb, in_=g_out_d[:, :])

        # ---- phase 0: weight-norm scales -> bf16 weights ----
        # norms -> [1,odim] psum row -> DRAM -> read back partition-transposed
        # [128, odim/128] (sqrt/recip/g-mul run 128-wide) -> PE-transpose so the
        # write-back DMA is w_ contiguous 256B rows (not a 2B element scatter)
        # -> broadcast-read [128, odim] bf16 -> scale the bf16 weights in place.
        def fold_weight_norm(w_bf, g_d, odim, use_barrier):
            w_ = odim // P
            with tc.tile_pool(name="wnorm", bufs=1) as wn, \
                 tc.tile_pool(name="wnps", bufs=2, space="PSUM") as wnps, \
                 tc.tile_pool(name="wdr", bufs=1, space="DRAM") as wdr:
                wsq = wn.tile([P, KC, odim], F32R, tag="wsq")
                for kc in range(KC):
                    nc.scalar.square(wsq[:, kc, :], w_bf[:, kc, :])
                if use_barrier:
                    tc.strict_bb_all_engine_barrier()
                n2d = wdr.tile([1, odim], F32, tag="n2d")
                sd = wdr.tile([1, odim], BF16, tag="sd")
                for os_ in range(odim // 512):
                    ps_n2 = wnps.tile([1, 512], F32, tag="n2ps")
                    for kc in range(KC):
                        nc.tensor.matmul(
                            ps_n2,
                            lhsT=ones_f,
                            rhs=wsq[:, kc, os_ * 512:(os_ + 1) * 512],
                            start=(kc == 0), stop=(kc == KC - 1),
                        )
                    n2row = wn.tile([1, 512], F32, tag="n2row", bufs=2)
                    nc.scalar.copy(out=n2row, in_=ps_n2)
                    nc.gpsimd.dma_start(out=n2d[:, os_ * 512:(os_ + 1) * 512], in_=n2row)
                n2t = wn.tile([P, w_], F32, tag="n2t")
                nc.gpsimd.dma_start(out=n2t, in_=n2d[0, :].rearrange("(j p) -> p j", p=P))
                gt = wn.tile([P, w_], F32, tag="gt")
                nc.gpsimd.dma_start(out=gt, in_=g_d[0, :].rearrange("(j p) -> p j", p=P))
                nc.scalar.sqrt(n2t, n2t)
                nc.vector.reciprocal(n2t, n2t)
                st = wn.tile([P, w_], BF16, tag="st")
                nc.vector.tensor_mul(st, n2t, gt)
                stp = wnps.tile([P, P], BF16, tag="stp")
                nc.tensor.transpose(out=stp[0:w_, :], in_=st, identity=ident)
                st2 = wn.tile([P, P], BF16, tag="st2")
                nc.vector.tensor_copy(out=st2[0:w_, :], in_=stp[0:w_, :])
                nc.gpsimd.dma_start(out=sd[0, :].rearrange("(j p) -> j p", p=P),
                                    in_=st2[0:w_, :])
                sbc = wn.tile([P, odim], BF16, tag="sbc")
                a = sd[0, :]
                nc.gpsimd.dma_start(out=sbc, in_=bass.AP(tensor=a.tensor, offset=a.offset,
                                                         ap=[[0, P]] + list(a.ap)))
                for kc in range(KC):
                    nc.vector.tensor_mul(w_bf[:, kc, :], w_bf[:, kc, :], sbc)

        fold_weight_norm(wq_bf, g_qkv_d, O3, True)
        fold_weight_norm(wo_bf, g_out_d, C, True)

        # ---- persistent attention operands ----
        tc.strict_bb_all_engine_barrier()
        big = ctx.enter_context(tc.tile_pool(name="big", bufs=1))
        k_a = big.tile([P, KC, NH], BF16)       # k_hat, [c-chunk][n<2048]
        k_b = big.tile([P, KC, NH], BF16)       # k_hat, [c-chunk][n>=2048]
        q_sb = big.tile([P, KC, NH], BF16)      # q_hat, [c-chunk][i]
        v_a = big.tile([P, NCH // 2, C], BF16)  # v_hat^T, [j<16][c']
        v_b = big.tile([P, NCH // 2, C], BF16)  # v_hat^T, [j>=16][c']

        # ---- phase 1 + 2, emission-interleaved for cross-phase overlap ----
        x_re = x_bf_d[:, :].rearrange("(k p) n -> p k n", p=P)

        def phase1_chunk(nch, xpool, qkvps, tpps, sqp, qnp, rp):
            x_sb = xpool.tile([P, KC, P], BF16, tag="x_sb")
            nc.sync.dma_start(out=x_sb, in_=x_re[:, :, nch * P:(nch + 1) * P])
            ps = qkvps.tile([P, 3, 512], F32, tag="ps")
            for os_ in range(3):
                for kc in range(KC):
                    nc.tensor.matmul(
                        ps[:, os_, :],
                        lhsT=x_sb[:, kc, :],
                        rhs=wq_bf[:, kc, os_ * 512:(os_ + 1) * 512],
                        start=(kc == 0), stop=(kc == KC - 1),
                    )
            sq = sqp.tile([P, 3, 512], F32, tag="sq")
            ssum = rp.tile([P, 1], F32, tag="ssum")
            nc.scalar.activation(out=sq, in_=ps,
                                 func=mybir.ActivationFunctionType.Square,
                                 accum_out=ssum)
            r = rp.tile([P, 1], F32, tag="r")
            nc.scalar.activation(out=r, in_=ssum,
                                 func=mybir.ActivationFunctionType.Sqrt,
                                 scale=1.0 / O3)
            nc.vector.tensor_scalar_add(r, r, 1e-4)
            nc.vector.reciprocal(r, r)
            v_half = v_a if nch < NCH // 2 else v_b
            nc.vector.tensor_scalar_mul(v_half[:, nch % (NCH // 2), :], ps[:, 2, :], r)
            qn = qnp.tile([P, 2, 512], BF16, tag="qn")
            nc.vector.tensor_scalar_mul(qn, ps[:, 0:2, :], r)
            k_half = k_a if nch < QCH else k_b
            kcol = (nch % QCH) * P
            for cc in range(KC):
                tp = tpps.tile([P, P], BF16, tag="tp")
                nc.tensor.transpose(out=tp, in_=qn[:, 1, cc * P:(cc + 1) * P], identity=ident)
                nc.vector.tensor_copy(out=k_half[:, cc, kcol:kcol + P], in_=tp)
            if nch < QCH:
                for cc in range(KC):
                    tp = tpps.tile([P, P], BF16, tag="tp")
                    nc.tensor.transpose(out=tp, in_=qn[:, 0, cc * P:(cc + 1) * P], identity=ident)
                    nc.vector.tensor_copy(out=q_sb[:, cc, nch * P:(nch + 1) * P], in_=tp)

        def attn_j(isl, j, h_ps, den_ps, scp, expp):
            k_half = k_a if j < QCH else k_b
            v_half = v_a if j < NCH // 2 else v_b
            kcol = (j % QCH) * P
            sc = scp.tile([P, ISL], F32, tag="sc")
            for cc in range(KC):
                nc.tensor.matmul(
                    sc,
                    lhsT=k_half[:, cc, kcol:kcol + P],
                    rhs=q_sb[:, cc, isl * ISL:(isl + 1) * ISL],
                    start=(cc == 0), stop=(cc == KC - 1),
                )
            ex = expp.tile([P, ISL], BF16, tag="ex")
            nc.scalar.activation(out=ex, in_=sc,
                                 func=mybir.ActivationFunctionType.Exp,
                                 scale=float(C) ** -0.5)
            nc.tensor.matmul(
                den_ps,
                lhsT=ones_mat,
                rhs=ex,
                start=(j == 0), stop=(j == NCH - 1),
            )
            for a in range(4):
                nc.tensor.matmul(
                    h_ps[:, a, :],
                    lhsT=ex[:, a * P:(a + 1) * P],
                    rhs=v_half[:, j % (NCH // 2), :],
                    start=(j == 0), stop=(j == NCH - 1),
                )

        def attn_epilogue(isl, h_ps, den_ps, hps, tp2p, hnp, hcnp, xtp, outp, rp2):
            dencp = hnp.tile([P, ISL], F32, tag="dencp")
            nc.vector.tensor_copy(out=dencp, in_=den_ps)
            rdens = []
            for a in range(4):
                dtp = tp2p.tile([P, P], F32, tag="tp2")
                nc.tensor.transpose(out=dtp, in_=dencp[:, a * P:(a + 1) * P], identity=identf)
                rden = rp2.tile([P, 1], F32, tag="rden")
                nc.vector.reciprocal(rden, dtp[:, 0:1])
                nc.vector.tensor_scalar_mul(rden, rden, float(2.0 ** -0.5))
                rdens.append(rden)
            hn = hnp.tile([P, 4, 512], BF16, tag="hn")
            for a in range(4):
                nc.vector.tensor_copy(out=hn[:, a, :], in_=h_ps[:, a, :])
            po = hps.tile([P, 4, 512], F32, tag="hslot")
            for a in range(4):
                ich = isl * 4 + a
                hcn = hcnp.tile([P, KC, P], BF16, tag="hcn")
                for cc in range(KC):
                    tp = tp2p.tile([P, P], BF16, tag="tp2")
                    nc.tensor.transpose(out=tp, in_=hn[:, a, cc * P:(cc + 1) * P], identity=ident)
                    nc.vector.tensor_copy(out=hcn[:, cc, :], in_=tp)
                for cc in range(KC):
                    nc.tensor.matmul(
                        po[:, a, :],
                        lhsT=hcn[:, cc, :],
                        rhs=wo_bf[:, cc, :],
                        start=(cc == 0), stop=(cc == KC - 1),
                    )
                xt_sb = xtp.tile([P, C], F32, tag="xt_sb")
                nc.sync.dma_start(out=xt_sb, in_=xt_nc[ich * P:(ich + 1) * P, :])
                ob = outp.tile([P, C], F32, tag="ob")
                nc.vector.scalar_tensor_tensor(
                    out=ob, in0=po[:, a, :], scalar=rdens[a], in1=xt_sb,
                    op0=mybir.AluOpType.mult, op1=mybir.AluOpType.add,
                )
                nc.sync.dma_start(out=out_d[ich * P:(ich + 1) * P, :], in_=ob)

        # SBUF pools (live throughout)
        sb_pools = [
            tc.tile_pool(name="xp", bufs=3),
            tc.tile_pool(name="sqp", bufs=2),
            tc.tile_pool(name="qnp", bufs=2),
            tc.tile_pool(name="rp", bufs=4),
            tc.tile_pool(name="exp", bufs=3),
            tc.tile_pool(name="hn", bufs=2),
            tc.tile_pool(name="hcn", bufs=2),
            tc.tile_pool(name="xtp", bufs=2),
            tc.tile_pool(name="outp", bufs=2),
            tc.tile_pool(name="rp2", bufs=8),
        ]
        xpool, sqp, qnp, rp, expp, hnp, hcnp, xtp, outp, rp2 = [
            ctx.enter_context(p) for p in sb_pools]

        # sequential emission: full phase 1, then attention
        psA = tc.tile_pool(name="qkvpsA", bufs=2, space="PSUM")
        tpA = tc.tile_pool(name="tppsA", bufs=2, space="PSUM")
        pA, tA = psA.__enter__(), tpA.__enter__()
        for nch in range(NCH):
            phase1_chunk(nch, xpool, pA, tA, sqp, qnp, rp)
        tpA.__exit__(None, None, None)
        psA.__exit__(None, None, None)

        hps = ctx.enter_context(tc.tile_pool(name="hps", bufs=1, space="PSUM"))
        dps = ctx.enter_context(tc.tile_pool(name="dps", bufs=1, space="PSUM"))
        scpA = ctx.enter_context(tc.tile_pool(name="scA", bufs=1, space="PSUM"))
        scpB = ctx.enter_context(tc.tile_pool(name="scB", bufs=1, space="PSUM"))
        tp2p = ctx.enter_context(tc.tile_pool(name="tp2", bufs=1, space="PSUM"))
        for isl in range(NISL):
            h_ps = hps.tile([P, 4, 512], F32, tag="hslot")
            den_ps = dps.tile([P, ISL], F32, tag="den")
            for j in range(NCH):
                attn_j(isl, j, h_ps, den_ps, scpA if j % 2 == 0 else scpB, expp)
            attn_epilogue(isl, h_ps, den_ps, hps, tp2p, hnp, hcnp, xtp, outp, rp2)

    _split_waits(nc)
    return nc


_GRAPH = None


def kernel(**inputs):
    global _GRAPH, LAST_RESULT
    x = np.ascontiguousarray(np.asarray(inputs["x"], dtype=np.float32))
    v_qkv = np.ascontiguousarray(np.asarray(inputs["v_qkv"], dtype=np.float32))
    g_qkv = np.ascontiguousarray(np.asarray(inputs["g_qkv"], dtype=np.float32))
    v_out = np.ascontiguousarray(np.asarray(inputs["v_out"], dtype=np.float32))
    g_out = np.ascontiguousarray(np.asarray(inputs["g_out"], dtype=np.float32))

    xt = x.reshape(B, C, N)
    wqkvT = np.ascontiguousarray(v_qkv.T)
    woutT = np.ascontiguousarray(v_out.T)
    ident = np.eye(P, dtype=ml_dtypes.bfloat16)
    g_qkv2 = np.ascontiguousarray(g_qkv.reshape(1, O3))
    g_out2 = np.ascontiguousarray(g_out.reshape(1, C))
    rsqrt2 = np.float32(2.0 ** -0.5)

    in_maps = []
    for core in range(8):
        b, h = core // 2, core % 2
        if h == 0:
            x_perm = xt[b]
        else:
            x_perm = np.concatenate([xt[b][:, NH:], xt[b][:, :NH]], axis=1)
        x_perm = np.ascontiguousarray(x_perm)
        in_maps.append({
            "x_bf": x_perm.astype(ml_dtypes.bfloat16),
            "xt_nc": np.ascontiguousarray(x_perm[:, :NH].T * rsqrt2),
            "wqkvT": wqkvT.astype(ml_dtypes.bfloat16),
            "g_qkv": g_qkv2,
            "woutT": woutT.astype(ml_dtypes.bfloat16),
            "g_out": g_out2,
            "ident": ident,
            "identf": np.eye(P, dtype=np.float32),
            "ones_col": np.ones((P, 1), np.float32),
        })

    if _GRAPH is None:
        _GRAPH = build_graph()

    res = run_bass_kernel_spmd(_GRAPH, in_maps, core_ids=list(range(8)))
    LAST_RESULT = res

    out = np.empty((B, C, N), np.float32)
    for core in range(8):
        b, h = core // 2, core % 2
        out[b][:, h * NH:(h + 1) * NH] = res.results[core]["out"].T
    return out.reshape(B, C, 64, 64)



# revision 13
# speedup vs baseline: 1.2921x; 1.2921x over previous
"""ADM attention block (B=4, C=512, H=W=64) on 8 TRN2 NeuronCores.

Sharding: core = (b, half) = (core//2, core%2). Data-parallel over batch (4)
x query-halves (2), zero collectives. The query half is selected on the host
by permuting the N axis of x so "my" queries are always columns 0:2048.

v2: weight-norm folded into host preprocessing; all heavy matmuls run in
fp8e4 with perf_mode=DoubleRow (K=256 per instruction, 2x PE throughput);
h accumulates transposed ([c, i] in PSUM) so the epilogue needs no PE
transposes; the softmax denominator comes from one DoubleRow ones-matmul
per j-pair into a [128,512] PSUM tile (M=128 -> replicated across
partitions), normalized via a broadcast reciprocal multiply. Vector work is
load-balanced across DVE (nc.vector) and Pool (nc.gpsimd).

Numerics: w_qkv is host-scaled by S=16 before the fp8 cast (RMS divide is
scale-invariant; the post-sqrt eps is compensated exactly by adding S*eps).
exp carries a -4ln2 bias so fp8 ex stays below the TRN e4m3 max of 240;
the 2^-4 factor cancels in h/den. The residual path stays f32 end-to-end.
"""

import os
from contextlib import ExitStack

import numpy as np
import ml_dtypes

import concourse.bass as bass
import concourse.mybir as mybir
import concourse.tile as tile
from concourse.bass_utils import run_bass_kernel_spmd

B, C, N = 4, 512, 4096
NH = N // 2
P = 128
O3 = 3 * C             # 1536
NCH = N // P           # 32 n-chunks
QCH = NH // P          # 16 query chunks per core
T = NCH // 2           # 16 j-pairs (DoubleRow contracts 256 keys at once)
ISL = 512              # query i-slice
NISL = NH // ISL       # 4 i-slices
S = 16.0               # host weight scale for fp8
F32 = mybir.dt.float32
BF16 = mybir.dt.bfloat16
F8 = mybir.dt.float8e4
DR = mybir.MatmulPerfMode.DoubleRow
EXP_BIAS = -2.772588722239781  # -4*ln(2): keeps fp8 ex <= ~15 << 240

LAST_RESULT = None

_TPB_ENGINES = (
    mybir.EngineType.PE,
    mybir.EngineType.Activation,
    mybir.EngineType.DVE,
    mybir.EngineType.Pool,
    mybir.EngineType.SP,
)


def _split_waits(nc):
    """walrus on this image rejects >1 sem-wait on a TPB instruction. Hoist
    excess waits onto engine-local NoOps, each carrying one wait."""
    ctr = 0
    for fn in nc.m.functions:
        for blk in fn.blocks:
            new_insts = []
            for inst in blk.instructions:
                si = getattr(inst, "sync_info", None)
                eng = getattr(inst, "engine", None)
                if (
                    si is not None
                    and si.on_wait
                    and len(si.on_wait) > 1
                    and eng in _TPB_ENGINES
                ):
                    for sw in si.on_wait[:-1]:
                        ctr += 1
                        nop = mybir.InstNoOp(
                            name=f"wsplit-{ctr}", engine=eng, ins=[], outs=[],
                            sync_info=mybir.SyncInfo(on_wait=[sw], on_update=[]),
                        )
                        new_insts.append(nop)
                    inst.sync_info = mybir.SyncInfo(
                        on_wait=[si.on_wait[-1]], on_update=si.on_update,
                    )
                new_insts.append(inst)
            blk.instructions[:] = new_insts


def build_graph():
    nc = bass.Bass()

    x_pack_d = nc.declare_dram_parameter("x_pack", [P, 4, N], F8, isOutput=False)
    w_pack_d = nc.declare_dram_parameter("w_pack", [P, 2, 2, O3], F8, isOutput=False)
    wo_pack_d = nc.declare_dram_parameter("wo_pack", [P, 4, C], BF16, isOutput=False)
    ident_d = nc.declare_dram_parameter("ident_bf", [P, P], BF16, isOutput=False)
    xt_nc = nc.declare_dram_parameter("xt_nc", [NH, C], F32, isOutput=False)
    out_d = nc.declare_dram_parameter("out", [NH, C], F32, isOutput=True)

    with tile.TileContext(nc) as tc, ExitStack() as ctx:
        singles = ctx.enter_context(tc.tile_pool(name="singles", bufs=1))

        w_sb = singles.tile([P, 2, 2, O3], F8)
        nc.sync.dma_start(out=w_sb, in_=w_pack_d[:, :, :, :])
        wo_sb = singles.tile([P, 4, C], BF16)
        nc.sync.dma_start(out=wo_sb, in_=wo_pack_d[:, :, :])
        ident = singles.tile([P, P], BF16)
        nc.sync.dma_start(out=ident, in_=ident_d[:, :])
        ones2 = singles.tile([P, 2, P], F8)
        nc.vector.memset(ones2, 1.0)
        ebias = singles.tile([P, 1], F32)
        nc.vector.memset(ebias, EXP_BIAS)

        # persistent attention operands
        big = ctx.enter_context(tc.tile_pool(name="big", bufs=1))
        q_sb = big.tile([P, 4, NH], F8)    # q_hat^T: [c-chunk][i]
        k_sb = big.tile([P, 4, N], F8)     # k_hat^T: [c-chunk][j]
        v_sb = big.tile([P, T, 2, C], F8)  # v_hat:   [j-pair][plane][c]

        # ---- phase 1: QKV (fp8 DoubleRow) + RMS + operand builds ----
        with tc.tile_pool(name="xp", bufs=3) as xp, \
             tc.tile_pool(name="qkvps", bufs=2, space="PSUM") as qkvps, \
             tc.tile_pool(name="tpps", bufs=2, space="PSUM") as tpps, \
             tc.tile_pool(name="sqp", bufs=2) as sqp, \
             tc.tile_pool(name="rp", bufs=4) as rp, \
             tc.tile_pool(name="qnp", bufs=2) as qnp:
            for nch in range(NCH):
                x_sb = xp.tile([P, 4, P], F8, tag="x_sb")
                nc.sync.dma_start(out=x_sb, in_=x_pack_d[:, :, nch * P:(nch + 1) * P])
                ps = qkvps.tile([P, 3, 512], F32, tag="ps")
                for os_ in range(3):
                    for c2 in range(2):
                        nc.tensor.matmul(
                            ps[:, os_, :],
                            lhsT=x_sb[:, 2 * c2:2 * c2 + 2, :],
                            rhs=w_sb[:, c2, :, os_ * 512:(os_ + 1) * 512],
                            start=(c2 == 0), stop=(c2 == 1),
                            perf_mode=DR,
                        )
                sq = sqp.tile([P, 3, 512], BF16, tag="sq")
                ssum = rp.tile([P, 1], F32, tag="ssum")
                nc.scalar.activation(out=sq, in_=ps,
                                     func=mybir.ActivationFunctionType.Square,
                                     accum_out=ssum)
                # r = 1/(S*(rms + eps)); ssum = S^2 * sum(qkv^2)
                r = rp.tile([P, 1], F32, tag="r")
                nc.scalar.activation(out=r, in_=ssum,
                                     func=mybir.ActivationFunctionType.Sqrt,
                                     scale=1.0 / O3)
                nc.vector.tensor_scalar_add(r, r, S * 1e-4)
                nc.vector.reciprocal(r, r)
                # v_hat straight into the attention rhs layout
                nc.vector.tensor_scalar_mul(
                    v_sb[:, nch // 2, nch % 2, :], ps[:, 2, :], r)
                # q,k normalized to fp8, then PE-transposed to [c, n]
                qn = qnp.tile([P, 2, 512], BF16, tag="qn")
                nc.vector.tensor_scalar_mul(qn, ps[:, 0:2, :], r)
                for cc in range(4):
                    tp = tpps.tile([P, P], BF16, tag="tp")
                    nc.tensor.transpose(out=tp, in_=qn[:, 1, cc * P:(cc + 1) * P],
                                        identity=ident)
                    nc.vector.tensor_copy(out=k_sb[:, cc, nch * P:(nch + 1) * P], in_=tp)
                if nch < QCH:
                    for cc in range(4):
                        tp = tpps.tile([P, P], BF16, tag="tp")
                        nc.tensor.transpose(out=tp, in_=qn[:, 0, cc * P:(cc + 1) * P],
                                            identity=ident)
                        if cc % 2 == 0:
                            nc.scalar.copy(out=q_sb[:, cc, nch * P:(nch + 1) * P], in_=tp)
                        else:
                            nc.vector.tensor_copy(out=q_sb[:, cc, nch * P:(nch + 1) * P], in_=tp)

        # ---- phase 2: attention (fp8 DoubleRow), hT accumulation ----
        scp = ctx.enter_context(tc.tile_pool(name="scp", bufs=1, space="PSUM"))
        hps = ctx.enter_context(tc.tile_pool(name="hps", bufs=1, space="PSUM"))
        dps = ctx.enter_context(tc.tile_pool(name="dps", bufs=1, space="PSUM"))
        pop = ctx.enter_context(tc.tile_pool(name="pop", bufs=1, space="PSUM"))
        expp = ctx.enter_context(tc.tile_pool(name="expp", bufs=2))
        rdp = ctx.enter_context(tc.tile_pool(name="rdp", bufs=2))
        htp = ctx.enter_context(tc.tile_pool(name="htp", bufs=2))
        xtp = ctx.enter_context(tc.tile_pool(name="xtp", bufs=3))
        obp = ctx.enter_context(tc.tile_pool(name="obp", bufs=3))

        for isl in range(NISL):
            h_ps = hps.tile([P, 4, 512], F32, tag="h")
            den_ps = dps.tile([P, 512], F32, tag="den")
            for t in range(T):
                sc = scp.tile([P, 2, 512], F32, tag="sc")
                for pl in range(2):
                    j = 2 * t + pl
                    for c2 in range(2):
                        nc.tensor.matmul(
                            sc[:, pl, :],
                            lhsT=k_sb[:, 2 * c2:2 * c2 + 2, j * P:(j + 1) * P],
                            rhs=q_sb[:, 2 * c2:2 * c2 + 2, isl * ISL:(isl + 1) * ISL],
                            start=(c2 == 0), stop=(c2 == 1),
                            perf_mode=DR,
                        )
                ex = expp.tile([P, 2, 512], F8, tag="ex")
                nc.scalar.activation(out=ex, in_=sc,
                                     func=mybir.ActivationFunctionType.Exp,
                                     scale=float(C) ** -0.5, bias=ebias)
                nc.tensor.matmul(
                    den_ps, lhsT=ones2, rhs=ex,
                    start=(t == 0), stop=(t == T - 1), perf_mode=DR,
                )
                for cc in range(4):
                    nc.tensor.matmul(
                        h_ps[:, cc, :],
                        lhsT=v_sb[:, t, :, cc * P:(cc + 1) * P],
                        rhs=ex,
                        start=(t == 0), stop=(t == T - 1),
                        perf_mode=DR,
                    )
            # epilogue: hT/den, project through w_out, add residual
            rden = rdp.tile([P, 512], F32, tag="rden")
            nc.vector.reciprocal(rden, den_ps)
            hTn = htp.tile([P, 4, 512], BF16, tag="hTn")
            for cc in range(4):
                nc.vector.tensor_mul(hTn[:, cc, :], h_ps[:, cc, :], rden)
            for a in range(4):
                po = pop.tile([P, 512], F32, tag="po")
                for cc in range(4):
                    nc.tensor.matmul(
                        po,
                        lhsT=hTn[:, cc, a * P:(a + 1) * P],
                        rhs=wo_sb[:, cc, :],
                        start=(cc == 0), stop=(cc == 3),
                    )
                ich = isl * 4 + a
                xt_sb = xtp.tile([P, C], F32, tag="xt_sb")
                nc.sync.dma_start(out=xt_sb, in_=xt_nc[ich * P:(ich + 1) * P, :])
                ob = obp.tile([P, C], F32, tag="ob")
                nc.vector.tensor_add(ob, po, xt_sb)
                nc.sync.dma_start(out=out_d[ich * P:(ich + 1) * P, :], in_=ob)

    _split_waits(nc)
    return nc


_GRAPH = None


def _f8(a):
    return np.asarray(a, dtype=np.float32).astype(ml_dtypes.float8_e4m3)


def kernel(**inputs):
    global _GRAPH, LAST_RESULT
    x = np.ascontiguousarray(np.asarray(inputs["x"], dtype=np.float32))
    v_qkv = np.asarray(inputs["v_qkv"], dtype=np.float32)
    g_qkv = np.asarray(inputs["g_qkv"], dtype=np.float32)
    v_out = np.asarray(inputs["v_out"], dtype=np.float32)
    g_out = np.asarray(inputs["g_out"], dtype=np.float32)

    # weight norm on host
    w_qkv = (g_qkv[:, None] * v_qkv
             / np.linalg.norm(v_qkv.astype(np.float64), axis=1, keepdims=True)
             ).astype(np.float32)  # [3C, C]
    w_out = (g_out[:, None] * v_out
             / np.linalg.norm(v_out.astype(np.float64), axis=1, keepdims=True)
             ).astype(np.float32)  # [C, C]

    # [128 p, 2 c2, 2 pl, O3]: w_pack[p,c2,pl,o] = S * w_qkv[o, c2*256+pl*128+p]
    wq = (S * w_qkv.T).reshape(2, 2, P, O3)
    w_pack = _f8(np.ascontiguousarray(wq.transpose(2, 0, 1, 3)))
    # [128 p, 4 cc, C]: wo_pack[p,cc,o] = 2^-0.5 * w_out[o, cc*128+p]
    wo = (np.float32(2.0 ** -0.5) * w_out.T).reshape(4, P, C)
    wo_pack = np.ascontiguousarray(wo.transpose(1, 0, 2)).astype(ml_dtypes.bfloat16)
    ident_bf = np.eye(P, dtype=ml_dtypes.bfloat16)
    rsqrt2 = np.float32(2.0 ** -0.5)

    xt = x.reshape(B, C, N)
    in_maps = []
    for core in range(8):
        b, h = core // 2, core % 2
        if h == 0:
            x_perm = xt[b]
        else:
            x_perm = np.concatenate([xt[b][:, NH:], xt[b][:, :NH]], axis=1)
        x_perm = np.ascontiguousarray(x_perm)
        x_pack = np.ascontiguousarray(
            x_perm.reshape(4, P, N).transpose(1, 0, 2))  # [128, 4cc, N]
        in_maps.append({
            "x_pack": _f8(x_pack),
            "w_pack": w_pack,
            "wo_pack": wo_pack,
            "ident_bf": ident_bf,
            "xt_nc": np.ascontiguousarray(x_perm[:, :NH].T * rsqrt2),
        })

    if _GRAPH is None:
        _GRAPH = build_graph()

    res = run_bass_kernel_spmd(_GRAPH, in_maps, core_ids=list(range(8)))
    LAST_RESULT = res

    out = np.empty((B, C, N), np.float32)
    for core in range(8):
        b, h = core // 2, core % 2
        out[b][:, h * NH:(h + 1) * NH] = res.results[core]["out"].T
    return out.reshape(B, C, 64, 64)


# revision 17
# speedup vs baseline: 1.4663x; 1.1348x over previous
"""ADM attention block (B=4, C=512, H=W=64) on 8 TRN2 NeuronCores.

Sharding: core = (b, half) = (core//2, core%2). Data-parallel over batch (4)
x query-halves (2), zero collectives. The query half is selected on the host
by permuting the N axis of x so "my" queries are always columns 0:2048.

v2: weight-norm folded into host preprocessing; all heavy matmuls run in
fp8e4 with perf_mode=DoubleRow (K=256 per instruction, 2x PE throughput);
h accumulates transposed ([c, i] in PSUM) so the epilogue needs no PE
transposes; the softmax denominator comes from one DoubleRow ones-matmul
per j-pair into a [128,512] PSUM tile (M=128 -> replicated across
partitions), normalized via a broadcast reciprocal multiply. Vector work is
load-balanced across DVE (nc.vector) and Pool (nc.gpsimd).

Numerics: w_qkv is host-scaled by S=16 before the fp8 cast (RMS divide is
scale-invariant; the post-sqrt eps is compensated exactly by adding S*eps).
exp carries a -4ln2 bias so fp8 ex stays below the TRN e4m3 max of 240;
the 2^-4 factor cancels in h/den. The residual path stays f32 end-to-end.
"""

import os
from contextlib import ExitStack

import numpy as np
import ml_dtypes

import concourse.bass as bass
import concourse.mybir as mybir
import concourse.tile as tile
from concourse.bass_utils import run_bass_kernel_spmd

B, C, N = 4, 512, 4096
NH = N // 2
P = 128
O3 = 3 * C             # 1536
NCH = N // P           # 32 n-chunks
QCH = NH // P          # 16 query chunks per core
T = NCH // 2           # 16 j-pairs (DoubleRow contracts 256 keys at once)
ISL = 512              # query i-slice
NISL = NH // ISL       # 4 i-slices
S = 16.0               # host weight scale for fp8
F32 = mybir.dt.float32
BF16 = mybir.dt.bfloat16
F8 = mybir.dt.float8e4
DR = mybir.MatmulPerfMode.DoubleRow
EXP_BIAS = -2.772588722239781  # -4*ln(2): keeps fp8 ex <= ~15 << 240

LAST_RESULT = None

_TPB_ENGINES = (
    mybir.EngineType.PE,
    mybir.EngineType.Activation,
    mybir.EngineType.DVE,
    mybir.EngineType.Pool,
    mybir.EngineType.SP,
)


def _split_waits(nc):
    """walrus on this image rejects >1 sem-wait on a TPB instruction. Hoist
    excess waits onto engine-local NoOps, each carrying one wait."""
    ctr = 0
    for fn in nc.m.functions:
        for blk in fn.blocks:
            new_insts = []
            for inst in blk.instructions:
                si = getattr(inst, "sync_info", None)
                eng = getattr(inst, "engine", None)
                if (
                    si is not None
                    and si.on_wait
                    and len(si.on_wait) > 1
                    and eng in _TPB_ENGINES
                ):
                    for sw in si.on_wait[:-1]:
                        ctr += 1
                        nop = mybir.InstNoOp(
                            name=f"wsplit-{ctr}", engine=eng, ins=[], outs=[],
                            sync_info=mybir.SyncInfo(on_wait=[sw], on_update=[]),
                        )
                        new_insts.append(nop)
                    inst.sync_info = mybir.SyncInfo(
                        on_wait=[si.on_wait[-1]], on_update=si.on_update,
                    )
                new_insts.append(inst)
            blk.instructions[:] = new_insts


def build_graph():
    nc = bass.Bass()

    x_pack_d = nc.declare_dram_parameter("x_pack", [P, 4, N], F8, isOutput=False)
    w_pack_d = nc.declare_dram_parameter("w_pack", [P, 2, 2, O3], F8, isOutput=False)
    wo_pack_d = nc.declare_dram_parameter("wo_pack", [P, 4, C], BF16, isOutput=False)
    ident_d = nc.declare_dram_parameter("ident_bf", [P, P], BF16, isOutput=False)
    xt_nc = nc.declare_dram_parameter("xt_nc", [NH, C], F32, isOutput=False)
    out_d = nc.declare_dram_parameter("out", [NH, C], F32, isOutput=True)

    with tile.TileContext(nc) as tc, ExitStack() as ctx:
        singles = ctx.enter_context(tc.tile_pool(name="singles", bufs=1))

        w_sb = singles.tile([P, 2, 2, O3], F8)
        nc.sync.dma_start(out=w_sb, in_=w_pack_d[:, :, :, :])
        wo_sb = singles.tile([P, 4, C], BF16)
        nc.sync.dma_start(out=wo_sb, in_=wo_pack_d[:, :, :])
        ident = singles.tile([P, P], BF16)
        nc.sync.dma_start(out=ident, in_=ident_d[:, :])
        ones2 = singles.tile([P, 2, P], F8)
        nc.vector.memset(ones2, 1.0)
        ebias = singles.tile([P, 1], F32)
        nc.vector.memset(ebias, EXP_BIAS)

        # persistent attention operands
        big = ctx.enter_context(tc.tile_pool(name="big", bufs=1))
        q_sb = big.tile([P, 4, NH], F8)    # q_hat^T: [c-chunk][i]
        k_sb = big.tile([P, 4, N], F8)     # k_hat^T: [c-chunk][j]
        v_sb = big.tile([P, T, 2, C], F8)  # v_hat:   [j-pair][plane][c]

        # ---- phase 1: QKV (fp8 DoubleRow) + RMS + operand builds ----
        with tc.tile_pool(name="xp", bufs=3) as xp, \
             tc.tile_pool(name="qkvps", bufs=2, space="PSUM") as qkvps, \
             tc.tile_pool(name="tpps", bufs=2, space="PSUM") as tpps, \
             tc.tile_pool(name="sqp", bufs=2) as sqp, \
             tc.tile_pool(name="rp", bufs=4) as rp, \
             tc.tile_pool(name="qnp", bufs=2) as qnp:
            for nch in range(NCH):
                x_sb = xp.tile([P, 4, P], F8, tag="x_sb")
                nc.sync.dma_start(out=x_sb, in_=x_pack_d[:, :, nch * P:(nch + 1) * P])
                ps = qkvps.tile([P, 3, 512], F32, tag="ps")
                for os_ in range(3):
                    for c2 in range(2):
                        nc.tensor.matmul(
                            ps[:, os_, :],
                            lhsT=x_sb[:, 2 * c2:2 * c2 + 2, :],
                            rhs=w_sb[:, c2, :, os_ * 512:(os_ + 1) * 512],
                            start=(c2 == 0), stop=(c2 == 1),
                            perf_mode=DR,
                        )
                sq = sqp.tile([P, 3, 512], BF16, tag="sq")
                ssum = rp.tile([P, 1], F32, tag="ssum")
                nc.scalar.activation(out=sq, in_=ps,
                                     func=mybir.ActivationFunctionType.Square,
                                     accum_out=ssum)
                # r = 1/(S*(rms + eps)); ssum = S^2 * sum(qkv^2)
                r = rp.tile([P, 1], F32, tag="r")
                nc.scalar.activation(out=r, in_=ssum,
                                     func=mybir.ActivationFunctionType.Sqrt,
                                     scale=1.0 / O3)
                nc.vector.tensor_scalar_add(r, r, S * 1e-4)
                nc.vector.reciprocal(r, r)
                # v_hat straight into the attention rhs layout (ACT: Copy*r)
                nc.scalar.activation(out=v_sb[:, nch // 2, nch % 2, :],
                                     in_=ps[:, 2, :],
                                     func=mybir.ActivationFunctionType.Copy,
                                     scale=r)
                # q,k normalized to bf16, then PE-transposed to [c, n];
                # the q half is only needed for the first QCH chunks
                qn = qnp.tile([P, 2, 512], BF16, tag="qn")
                if nch < QCH:
                    nc.vector.tensor_scalar_mul(qn, ps[:, 0:2, :], r)
                else:
                    nc.vector.tensor_scalar_mul(qn[:, 1, :], ps[:, 1, :], r)
                for cc in range(4):
                    tp = tpps.tile([P, P], BF16, tag="tp")
                    nc.tensor.transpose(out=tp, in_=qn[:, 1, cc * P:(cc + 1) * P],
                                        identity=ident)
                    nc.vector.tensor_copy(out=k_sb[:, cc, nch * P:(nch + 1) * P], in_=tp)
                if nch < QCH:
                    for cc in range(4):
                        tp = tpps.tile([P, P], BF16, tag="tp")
                        nc.tensor.transpose(out=tp, in_=qn[:, 0, cc * P:(cc + 1) * P],
                                            identity=ident)
                        if cc % 2 == 0:
                            nc.scalar.copy(out=q_sb[:, cc, nch * P:(nch + 1) * P], in_=tp)
                        else:
                            nc.vector.tensor_copy(out=q_sb[:, cc, nch * P:(nch + 1) * P], in_=tp)

        # ---- phase 2: attention (fp8 DoubleRow), hT accumulation ----
        scp = ctx.enter_context(tc.tile_pool(name="scp", bufs=1, space="PSUM"))
        hps = ctx.enter_context(tc.tile_pool(name="hps", bufs=1, space="PSUM"))
        dps = ctx.enter_context(tc.tile_pool(name="dps", bufs=1, space="PSUM"))
        pop = ctx.enter_context(tc.tile_pool(name="pop", bufs=1, space="PSUM"))
        expp = ctx.enter_context(tc.tile_pool(name="expp", bufs=2))
        rdp = ctx.enter_context(tc.tile_pool(name="rdp", bufs=2))
        htp = ctx.enter_context(tc.tile_pool(name="htp", bufs=2))
        xtp = ctx.enter_context(tc.tile_pool(name="xtp", bufs=3))
        obp = ctx.enter_context(tc.tile_pool(name="obp", bufs=3))

        tiles = {}  # isl -> (h_ps, den_ps), allocated lazily at first acc

        def scores_exp(isl, t):
            sc = scp.tile([P, 2, 512], F32, tag="sc")
            for pl in range(2):
                j = 2 * t + pl
                for c2 in range(2):
                    nc.tensor.matmul(
                        sc[:, pl, :],
                        lhsT=k_sb[:, 2 * c2:2 * c2 + 2, j * P:(j + 1) * P],
                        rhs=q_sb[:, 2 * c2:2 * c2 + 2, isl * ISL:(isl + 1) * ISL],
                        start=(c2 == 0), stop=(c2 == 1),
                        perf_mode=DR,
                    )
            ex = expp.tile([P, 2, 512], F8, tag="ex")
            nc.scalar.activation(out=ex, in_=sc,
                                 func=mybir.ActivationFunctionType.Exp,
                                 scale=float(C) ** -0.5, bias=ebias)
            return ex

        def acc_h_den(isl, t, ex):
            if isl not in tiles:
                h_t = hps.tile([P, 4, 512], F32, tag="h")
                den_t = dps.tile([P, 512], F32, tag="den")
                tiles[isl] = (h_t, den_t)
            h_ps, den_ps = tiles[isl]
            nc.tensor.matmul(
                den_ps, lhsT=ones2, rhs=ex,
                start=(t == 0), stop=(t == T - 1), perf_mode=DR,
            )
            for cc in range(4):
                nc.tensor.matmul(
                    h_ps[:, cc, :],
                    lhsT=v_sb[:, t, :, cc * P:(cc + 1) * P],
                    rhs=ex,
                    start=(t == 0), stop=(t == T - 1),
                    perf_mode=DR,
                )

        def epilogue(isl):
            h_ps, den_ps = tiles.pop(isl)
            rden = rdp.tile([P, 512], F32, tag="rden")
            nc.vector.reciprocal(rden, den_ps)
            hTn = htp.tile([P, 4, 512], BF16, tag="hTn")
            for cc in range(4):
                nc.vector.tensor_mul(hTn[:, cc, :], h_ps[:, cc, :], rden)
            for a in range(4):
                po = pop.tile([P, 512], F32, tag="po")
                for cc in range(4):
                    nc.tensor.matmul(
                        po,
                        lhsT=hTn[:, cc, a * P:(a + 1) * P],
                        rhs=wo_sb[:, cc, :],
                        start=(cc == 0), stop=(cc == 3),
                    )
                ich = isl * 4 + a
                xt_sb = xtp.tile([P, C], F32, tag="xt_sb")
                nc.sync.dma_start(out=xt_sb, in_=xt_nc[ich * P:(ich + 1) * P, :])
                ob = obp.tile([P, C], F32, tag="ob")
                nc.vector.tensor_add(ob, po, xt_sb)
                nc.sync.dma_start(out=out_d[ich * P:(ich + 1) * P, :], in_=ob)

        # flat software pipeline over all (isl, t) pairs: emit scores(t+1)
        # before h/den(t) so the PE computes scores while ACT runs exp; the
        # per-isl epilogue is emitted right after its last h/den lands.
        pairs = [(isl, t) for isl in range(NISL) for t in range(T)]
        prev = None
        for cur in pairs:
            ex_cur = scores_exp(*cur)
            if prev is not None:
                acc_h_den(prev[0], prev[1], ex_prev)
                if prev[1] == T - 1:
                    epilogue(prev[0])
            prev, ex_prev = cur, ex_cur
        acc_h_den(prev[0], prev[1], ex_prev)
        epilogue(prev[0])

    _split_waits(nc)
    return nc


_GRAPH = None


def _f8(a):
    return np.asarray(a, dtype=np.float32).astype(ml_dtypes.float8_e4m3)


def kernel(**inputs):
    global _GRAPH, LAST_RESULT
    x = np.ascontiguousarray(np.asarray(inputs["x"], dtype=np.float32))
    v_qkv = np.asarray(inputs["v_qkv"], dtype=np.float32)
    g_qkv = np.asarray(inputs["g_qkv"], dtype=np.float32)
    v_out = np.asarray(inputs["v_out"], dtype=np.float32)
    g_out = np.asarray(inputs["g_out"], dtype=np.float32)

    # weight norm on host
    w_qkv = (g_qkv[:, None] * v_qkv
             / np.linalg.norm(v_qkv.astype(np.float64), axis=1, keepdims=True)
             ).astype(np.float32)  # [3C, C]
    w_out = (g_out[:, None] * v_out
             / np.linalg.norm(v_out.astype(np.float64), axis=1, keepdims=True)
             ).astype(np.float32)  # [C, C]

    # [128 p, 2 c2, 2 pl, O3]: w_pack[p,c2,pl,o] = S * w_qkv[o, c2*256+pl*128+p]
    wq = (S * w_qkv.T).reshape(2, 2, P, O3)
    w_pack = _f8(np.ascontiguousarray(wq.transpose(2, 0, 1, 3)))
    # [128 p, 4 cc, C]: wo_pack[p,cc,o] = 2^-0.5 * w_out[o, cc*128+p]
    wo = (np.float32(2.0 ** -0.5) * w_out.T).reshape(4, P, C)
    wo_pack = np.ascontiguousarray(wo.transpose(1, 0, 2)).astype(ml_dtypes.bfloat16)
    ident_bf = np.eye(P, dtype=ml_dtypes.bfloat16)
    rsqrt2 = np.float32(2.0 ** -0.5)

    xt = x.reshape(B, C, N)
    in_maps = []
    for core in range(8):
        b, h = core // 2, core % 2
        if h == 0:
            x_perm = xt[b]
        else:
            x_perm = np.concatenate([xt[b][:, NH:], xt[b][:, :NH]], axis=1)
        x_perm = np.ascontiguousarray(x_perm)
        x_pack = np.ascontiguousarray(
            x_perm.reshape(4, P, N).transpose(1, 0, 2))  # [128, 4cc, N]
        in_maps.append({
            "x_pack": _f8(x_pack),
            "w_pack": w_pack,
            "wo_pack": wo_pack,
            "ident_bf": ident_bf,
            "xt_nc": np.ascontiguousarray(x_perm[:, :NH].T * rsqrt2),
        })

    if _GRAPH is None:
        _GRAPH = build_graph()

    res = run_bass_kernel_spmd(_GRAPH, in_maps, core_ids=list(range(8)))
    LAST_RESULT = res

    out = np.empty((B, C, N), np.float32)
    for core in range(8):
        b, h = core // 2, core % 2
        out[b][:, h * NH:(h + 1) * NH] = res.results[core]["out"].T
    return out.reshape(B, C, 64, 64)


# revision 21
# speedup vs baseline: 1.6653x; 1.1357x over previous
"""ADM attention block (B=4, C=512, H=W=64) on 8 TRN2 NeuronCores.

Sharding: core = (b, half) = (core//2, core%2). Data-parallel over batch (4)
x query-halves (2), zero collectives. The query half is selected on the host
by permuting the N axis of x so "my" queries are always columns 0:2048.

v2: weight-norm folded into host preprocessing; all heavy matmuls run in
fp8e4 with perf_mode=DoubleRow (K=256 per instruction, 2x PE throughput);
h accumulates transposed ([c, i] in PSUM) so the epilogue needs no PE
transposes; the softmax denominator comes from one DoubleRow ones-matmul
per j-pair into a [128,512] PSUM tile (M=128 -> replicated across
partitions), normalized via a broadcast reciprocal multiply. Vector work is
load-balanced across DVE (nc.vector) and Pool (nc.gpsimd).

Numerics: w_qkv is host-scaled by S=16 before the fp8 cast (RMS divide is
scale-invariant; the post-sqrt eps is compensated exactly by adding S*eps).
exp carries a -4ln2 bias so fp8 ex stays below the TRN e4m3 max of 240;
the 2^-4 factor cancels in h/den. The residual path stays f32 end-to-end.
"""

import os
from contextlib import ExitStack

import numpy as np
import ml_dtypes

import concourse.bass as bass
import concourse.mybir as mybir
import concourse.tile as tile
from concourse.bass_utils import run_bass_kernel_spmd

B, C, N = 4, 512, 4096
NH = N // 2
P = 128
O3 = 3 * C             # 1536
NCH = N // P           # 32 n-chunks
QCH = NH // P          # 16 query chunks per core
T = NCH // 2           # 16 j-pairs (DoubleRow contracts 256 keys at once)
ISL = 512              # query i-slice
NISL = NH // ISL       # 4 i-slices
S = 16.0               # host weight scale for fp8
F32 = mybir.dt.float32
BF16 = mybir.dt.bfloat16
F8 = mybir.dt.float8e4
DR = mybir.MatmulPerfMode.DoubleRow
EXP_BIAS = -2.772588722239781  # -4*ln(2): keeps fp8 ex <= ~15 << 240

LAST_RESULT = None

_TPB_ENGINES = (
    mybir.EngineType.PE,
    mybir.EngineType.Activation,
    mybir.EngineType.DVE,
    mybir.EngineType.Pool,
    mybir.EngineType.SP,
)


def _split_waits(nc):
    """walrus on this image rejects >1 sem-wait on a TPB instruction. Hoist
    excess waits onto engine-local NoOps, each carrying one wait."""
    ctr = 0
    for fn in nc.m.functions:
        for blk in fn.blocks:
            new_insts = []
            for inst in blk.instructions:
                si = getattr(inst, "sync_info", None)
                eng = getattr(inst, "engine", None)
                if (
                    si is not None
                    and si.on_wait
                    and len(si.on_wait) > 1
                    and eng in _TPB_ENGINES
                ):
                    for sw in si.on_wait[:-1]:
                        ctr += 1
                        nop = mybir.InstNoOp(
                            name=f"wsplit-{ctr}", engine=eng, ins=[], outs=[],
                            sync_info=mybir.SyncInfo(on_wait=[sw], on_update=[]),
                        )
                        new_insts.append(nop)
                    inst.sync_info = mybir.SyncInfo(
                        on_wait=[si.on_wait[-1]], on_update=si.on_update,
                    )
                new_insts.append(inst)
            blk.instructions[:] = new_insts


def build_graph():
    nc = bass.Bass()

    x_pack_d = nc.declare_dram_parameter("x_pack", [P, 4, N], F8, isOutput=False)
    w_pack_d = nc.declare_dram_parameter("w_pack", [P, 2, 2, O3], F8, isOutput=False)
    wo_pack_d = nc.declare_dram_parameter("wo_pack", [P, 4, C], BF16, isOutput=False)
    ident_d = nc.declare_dram_parameter("ident_bf", [P, P], BF16, isOutput=False)
    xt_nc = nc.declare_dram_parameter("xt_nc", [NH, C], F32, isOutput=False)
    out_d = nc.declare_dram_parameter("out", [NH, C], F32, isOutput=True)

    with tile.TileContext(nc) as tc, ExitStack() as ctx:
        singles = ctx.enter_context(tc.tile_pool(name="singles", bufs=1))

        w_sb = singles.tile([P, 2, 2, O3], F8)
        nc.sync.dma_start(out=w_sb, in_=w_pack_d[:, :, :, :])
        wo_sb = singles.tile([P, 4, C], BF16)
        nc.sync.dma_start(out=wo_sb, in_=wo_pack_d[:, :, :])
        ident = singles.tile([P, P], BF16)
        nc.sync.dma_start(out=ident, in_=ident_d[:, :])
        ones2 = singles.tile([P, 2, P], F8)
        nc.vector.memset(ones2, 1.0)
        ebias = singles.tile([P, 1], F32)
        nc.vector.memset(ebias, EXP_BIAS)

        # persistent attention operands
        big = ctx.enter_context(tc.tile_pool(name="big", bufs=1))
        q_sb = big.tile([P, 4, NH], F8)    # q_hat^T: [c-chunk][i]
        k_sb = big.tile([P, 4, N], F8)     # k_hat^T: [c-chunk][j]
        v_sb = big.tile([P, T, 2, C], F8)  # v_hat:   [j-pair][plane][c]

        # ---- phase 1: QKV (fp8 DoubleRow) + RMS + operand builds ----
        with tc.tile_pool(name="xp", bufs=3) as xp, \
             tc.tile_pool(name="qkvps", bufs=2, space="PSUM") as qkvps, \
             tc.tile_pool(name="tpps", bufs=2, space="PSUM") as tpps, \
             tc.tile_pool(name="sqp", bufs=2) as sqp, \
             tc.tile_pool(name="rp", bufs=4) as rp, \
             tc.tile_pool(name="qnp", bufs=2) as qnp:
            for nch in range(NCH):
                x_sb = xp.tile([P, 4, P], F8, tag="x_sb")
                nc.sync.dma_start(out=x_sb, in_=x_pack_d[:, :, nch * P:(nch + 1) * P])
                ps = qkvps.tile([P, 3, 512], F32, tag="ps")
                for os_ in range(3):
                    for c2 in range(2):
                        nc.tensor.matmul(
                            ps[:, os_, :],
                            lhsT=x_sb[:, 2 * c2:2 * c2 + 2, :],
                            rhs=w_sb[:, c2, :, os_ * 512:(os_ + 1) * 512],
                            start=(c2 == 0), stop=(c2 == 1),
                            perf_mode=DR,
                        )
                sq = sqp.tile([P, 3, 512], BF16, tag="sq")
                ssum = rp.tile([P, 1], F32, tag="ssum")
                nc.scalar.activation(out=sq, in_=ps,
                                     func=mybir.ActivationFunctionType.Square,
                                     accum_out=ssum)
                # r = 1/(S*(rms + eps)); ssum = S^2 * sum(qkv^2)
                r = rp.tile([P, 1], F32, tag="r")
                nc.scalar.activation(out=r, in_=ssum,
                                     func=mybir.ActivationFunctionType.Sqrt,
                                     scale=1.0 / O3)
                nc.vector.tensor_scalar_add(r, r, S * 1e-4)
                nc.vector.reciprocal(r, r)
                # v_hat straight into the attention rhs layout
                nc.vector.tensor_scalar_mul(
                    v_sb[:, nch // 2, nch % 2, :], ps[:, 2, :], r)
                # q,k normalized to bf16, then PE-transposed to [c, n];
                # the q half is only needed for the first QCH chunks
                qn = qnp.tile([P, 2, 512], BF16, tag="qn")
                if nch < QCH:
                    nc.vector.tensor_scalar_mul(qn, ps[:, 0:2, :], r)
                else:
                    nc.vector.tensor_scalar_mul(qn[:, 1, :], ps[:, 1, :], r)
                tpk = tpps.tile([P, 4, P], BF16, tag="tp")
                for cc in range(4):
                    nc.tensor.transpose(out=tpk[:, cc, :], in_=qn[:, 1, cc * P:(cc + 1) * P],
                                        identity=ident)
                nc.vector.tensor_copy(out=k_sb[:, :, nch * P:(nch + 1) * P], in_=tpk)
                if nch < QCH:
                    tpq = tpps.tile([P, 4, P], BF16, tag="tp")
                    for cc in range(4):
                        nc.tensor.transpose(out=tpq[:, cc, :], in_=qn[:, 0, cc * P:(cc + 1) * P],
                                            identity=ident)
                    nc.scalar.copy(out=q_sb[:, :, nch * P:(nch + 1) * P], in_=tpq)

        # ---- phase 2: attention (fp8 DoubleRow), hT accumulation ----
        scp = ctx.enter_context(tc.tile_pool(name="scp", bufs=1, space="PSUM"))
        hps = ctx.enter_context(tc.tile_pool(name="hps", bufs=1, space="PSUM"))
        dps = ctx.enter_context(tc.tile_pool(name="dps", bufs=1, space="PSUM"))
        pop = ctx.enter_context(tc.tile_pool(name="pop", bufs=1, space="PSUM"))
        expp = ctx.enter_context(tc.tile_pool(name="expp", bufs=2))
        rdp = ctx.enter_context(tc.tile_pool(name="rdp", bufs=2))
        htp = ctx.enter_context(tc.tile_pool(name="htp", bufs=2))
        xtp = ctx.enter_context(tc.tile_pool(name="xtp", bufs=3))
        obp = ctx.enter_context(tc.tile_pool(name="obp", bufs=3))

        tiles = {}  # isl -> (h_ps, den_ps), allocated lazily at first acc

        def scores_exp(isl, t):
            sc = scp.tile([P, 2, 512], F32, tag="sc")
            for pl in range(2):
                j = 2 * t + pl
                for c2 in range(2):
                    nc.tensor.matmul(
                        sc[:, pl, :],
                        lhsT=k_sb[:, 2 * c2:2 * c2 + 2, j * P:(j + 1) * P],
                        rhs=q_sb[:, 2 * c2:2 * c2 + 2, isl * ISL:(isl + 1) * ISL],
                        start=(c2 == 0), stop=(c2 == 1),
                        perf_mode=DR,
                    )
            ex = expp.tile([P, 2, 512], F8, tag="ex")
            nc.scalar.activation(out=ex, in_=sc,
                                 func=mybir.ActivationFunctionType.Exp,
                                 scale=float(C) ** -0.5, bias=ebias)
            return ex

        def acc_h_den(isl, t, ex):
            if isl not in tiles:
                h_t = hps.tile([P, 4, 512], F32, tag="h")
                den_t = dps.tile([P, 512], F32, tag="den")
                tiles[isl] = (h_t, den_t)
            h_ps, den_ps = tiles[isl]
            nc.tensor.matmul(
                den_ps, lhsT=ones2, rhs=ex,
                start=(t == 0), stop=(t == T - 1), perf_mode=DR,
            )
            for cc in range(4):
                nc.tensor.matmul(
                    h_ps[:, cc, :],
                    lhsT=v_sb[:, t, :, cc * P:(cc + 1) * P],
                    rhs=ex,
                    start=(t == 0), stop=(t == T - 1),
                    perf_mode=DR,
                )

        def epilogue(isl):
            h_ps, den_ps = tiles.pop(isl)
            rden = rdp.tile([P, 512], F32, tag="rden")
            nc.vector.reciprocal(rden, den_ps)
            hTn = htp.tile([P, 4, 512], BF16, tag="hTn")
            for cc in range(4):
                nc.vector.tensor_mul(hTn[:, cc, :], h_ps[:, cc, :], rden)
            for a in range(4):
                po = pop.tile([P, 512], F32, tag="po")
                for cc in range(4):
                    nc.tensor.matmul(
                        po,
                        lhsT=hTn[:, cc, a * P:(a + 1) * P],
                        rhs=wo_sb[:, cc, :],
                        start=(cc == 0), stop=(cc == 3),
                    )
                ich = isl * 4 + a
                xt_sb = xtp.tile([P, C], F32, tag="xt_sb")
                nc.sync.dma_start(out=xt_sb, in_=xt_nc[ich * P:(ich + 1) * P, :])
                ob = obp.tile([P, C], F32, tag="ob")
                nc.vector.tensor_add(ob, po, xt_sb)
                nc.sync.dma_start(out=out_d[ich * P:(ich + 1) * P, :], in_=ob)

        # flat software pipeline over all (isl, t) pairs: emit scores(t+1)
        # before h/den(t) so the PE computes scores while ACT runs exp; the
        # per-isl epilogue is emitted right after its last h/den lands.
        pairs = [(isl, t) for isl in range(NISL) for t in range(T)]
        prev = None
        for cur in pairs:
            ex_cur = scores_exp(*cur)
            if prev is not None:
                acc_h_den(prev[0], prev[1], ex_prev)
                if prev[1] == T - 1:
                    epilogue(prev[0])
            prev, ex_prev = cur, ex_cur
        acc_h_den(prev[0], prev[1], ex_prev)
        epilogue(prev[0])

    _split_waits(nc)
    return nc


_GRAPH = None


def _f8(a):
    return np.asarray(a, dtype=np.float32).astype(ml_dtypes.float8_e4m3)


def kernel(**inputs):
    global _GRAPH, LAST_RESULT
    x = np.ascontiguousarray(np.asarray(inputs["x"], dtype=np.float32))
    v_qkv = np.asarray(inputs["v_qkv"], dtype=np.float32)
    g_qkv = np.asarray(inputs["g_qkv"], dtype=np.float32)
    v_out = np.asarray(inputs["v_out"], dtype=np.float32)
    g_out = np.asarray(inputs["g_out"], dtype=np.float32)

    # weight norm on host
    w_qkv = (g_qkv[:, None] * v_qkv
             / np.linalg.norm(v_qkv.astype(np.float64), axis=1, keepdims=True)
             ).astype(np.float32)  # [3C, C]
    w_out = (g_out[:, None] * v_out
             / np.linalg.norm(v_out.astype(np.float64), axis=1, keepdims=True)
             ).astype(np.float32)  # [C, C]

    # [128 p, 2 c2, 2 pl, O3]: w_pack[p,c2,pl,o] = S * w_qkv[o, c2*256+pl*128+p]
    wq = (S * w_qkv.T).reshape(2, 2, P, O3)
    w_pack = _f8(np.ascontiguousarray(wq.transpose(2, 0, 1, 3)))
    # [128 p, 4 cc, C]: wo_pack[p,cc,o] = 2^-0.5 * w_out[o, cc*128+p]
    wo = (np.float32(2.0 ** -0.5) * w_out.T).reshape(4, P, C)
    wo_pack = np.ascontiguousarray(wo.transpose(1, 0, 2)).astype(ml_dtypes.bfloat16)
    ident_bf = np.eye(P, dtype=ml_dtypes.bfloat16)
    rsqrt2 = np.float32(2.0 ** -0.5)

    xt = x.reshape(B, C, N)
    in_maps = []
    for core in range(8):
        b, h = core // 2, core % 2
        if h == 0:
            x_perm = xt[b]
        else:
            x_perm = np.concatenate([xt[b][:, NH:], xt[b][:, :NH]], axis=1)
        x_perm = np.ascontiguousarray(x_perm)
        x_pack = np.ascontiguousarray(
            x_perm.reshape(4, P, N).transpose(1, 0, 2))  # [128, 4cc, N]
        in_maps.append({
            "x_pack": _f8(x_pack),
            "w_pack": w_pack,
            "wo_pack": wo_pack,
            "ident_bf": ident_bf,
            "xt_nc": np.ascontiguousarray(x_perm[:, :NH].T * rsqrt2),
        })

    if _GRAPH is None:
        _GRAPH = build_graph()

    res = run_bass_kernel_spmd(_GRAPH, in_maps, core_ids=list(range(8)))
    LAST_RESULT = res

    out = np.empty((B, C, N), np.float32)
    for core in range(8):
        b, h = core // 2, core % 2
        out[b][:, h * NH:(h + 1) * NH] = res.results[core]["out"].T
    return out.reshape(B, C, 64, 64)
